# revision 14
# baseline (speedup 1.0000x reference)
"""Bottleneck residual block (1x1 -> 3x3 -> 1x1 conv + BN + residual) on 8 NeuronCores.

Strategy: data-parallel over batch (16 images -> 2 per core), software
pipelined at image granularity: img0 flows stage1->stage2->stage3 while
img1's stage1/2 matmuls fill the PE, so img0's stage-3 epilogues hide
under img1 compute.

  - one upload xb = f16(x + beta3*2^q3): stage 1 consumes it with a
    host-corrected bias (so beta3 costs nothing), stage 3 adds it as the
    pre-biased residual.
  - stage 1 runs f16 (K=1024), stages 2+3 fp8e4m3 DoubleRow (K=256/pass).
  - stages 1/2 epilogue: one ACT op (or a 2-op DVE chain) per (m, img):
    Relu(psum*a+b) -> fp8 direct.
  - stage 3 epilogue: one DVE scalar_tensor_tensor per (m, img):
    (psum*a3 + xb) -> int8 with RNE + saturation (exact round + clip at
    127); final relu done host-side on the int8 results (max(out,0)).
    A couple of tail units run ACT-drain + GpSimd f16 add + DVE convert
    to spread the load.
Intermediate BN rounding (round-to-nearest of bn1/bn2) is skipped: the
error enters the next conv scaled by alpha*2^-12 (~1e-5); end-to-end rel
err is ~1.6e-3, far under the 2e-2 gate (validated against reference).

DMA triggers are split across the Sync and ACT HWDGE queues (trigger
issue costs ~0.65us each, which otherwise paces the input stream); a
dummy ACT op preloads the activation table; dummy warm-up matmuls ramp
the PE HAM clock to 2.4GHz during the initial DMA window.

Shapes hardcoded for N=16, Cin=Cout=1024, width=256, H=W=28.
"""

import numpy as np
import ml_dtypes

BF16 = ml_dtypes.bfloat16
FP8 = ml_dtypes.float8_e4m3
F16 = np.float16

N_CORES = 8
N_PER_CORE = 2          # images per core
HW1 = 28 * 28           # 784 spatial positions per image
F = N_PER_CORE * HW1    # 1568 free-dim elements per core
FB = 392                # matmul free-dim block (14 rows of 28)

N_WARMUP = 22           # dummy matmuls to ramp the PE clock

# stage-3 units using the ACT+GpSimd+DVE-convert spread (img1 tail only)
S3_GP_UNITS = {(3, 0), (5, 0)}

_CACHE = {}


def _build():
    import concourse.bacc as bacc
    import concourse.mybir as mybir
    import concourse.tile as tile

    dt = mybir.dt
    f32, f16, i8, fp8 = dt.float32, dt.float16, dt.int8, dt.float8e4
    Alu = mybir.AluOpType
    Act = mybir.ActivationFunctionType
    DR = mybir.MatmulPerfMode.DoubleRow

    nc = bacc.Bacc("TRN2", target_bir_lowering=False, debug=False,
                   num_devices=N_CORES, enable_partition_id=False)

    x8_d = nc.dram_tensor("x8", [128, 8, HW1], i8, kind="ExternalInput")
    xb_d = nc.dram_tensor("xb", [128, 8, HW1], f16, kind="ExternalInput")
    w1_d = nc.dram_tensor("w1", [128, 16, 128], f16, kind="ExternalInput")
    w2_d = nc.dram_tensor("w2", [128, 18, 2, 128], fp8, kind="ExternalInput")
    w3_d = nc.dram_tensor("w3", [128, 8, 2, 128], fp8, kind="ExternalInput")
    vec_d = nc.dram_tensor("vec", [128, 24], f32, kind="ExternalInput")
    out_d = nc.dram_tensor("out", [128, 8, F], i8, kind="ExternalOutput")

    with tile.TileContext(nc) as tc:
        with (
            tc.tile_pool(name="persist", bufs=1) as pp,
            tc.tile_pool(name="stage", bufs=4) as sp,
            tc.tile_pool(name="psum", bufs=1, space="PSUM") as psp,
        ):
            # ---- persistent SBUF tensors ----
            warm = pp.tile([128, 128], f16, tag="warm", name="warm")
            nc.vector.memset(warm[:], 0.0)
            # preload the ACT function table before it's needed
            twu = sp.tile([128, 1], f32, tag="twu", name="twu")
            nc.scalar.activation(twu[:], warm[:, 0:1], Act.Relu)

            xb_sb = pp.tile([128, 2, 8, HW1], f16, tag="xb", name="xb")
            x8_sb = pp.tile([128, 8, HW1], i8, tag="x8", name="x8")
            w1_sb = pp.tile([128, 16, 128], f16, tag="w1", name="w1")
            vec_sb = pp.tile([128, 24], f32, tag="vec", name="vec")
            w2_sb = pp.tile([128, 18, 2, 128], fp8, tag="w2", name="w2")
            w3_sb = pp.tile([128, 8, 2, 128], fp8, tag="w3", name="w3")

            # input DMA: one queue, strictly in stage-1 consumption order
            # (concurrent transfers steal bandwidth from the critical chunk);
            # only the tiny vec rides the ACT queue.
            nc.sync.dma_start(w1_sb[:, 0:4], w1_d[:, 0:4])
            nc.sync.dma_start(x8_sb[:, 0:2], x8_d[:, 0:2])
            nc.scalar.dma_start(vec_sb[:], vec_d[:])
            nc.sync.dma_start(w1_sb[:, 4:10], w1_d[:, 4:10])
            nc.sync.dma_start(x8_sb[:, 2:5], x8_d[:, 2:5])
            nc.sync.dma_start(w1_sb[:, 10:16], w1_d[:, 10:16])
            nc.sync.dma_start(x8_sb[:, 5:8], x8_d[:, 5:8])
            nc.sync.dma_start(w2_sb[:], w2_d[:])
            nc.sync.dma_start(xb_sb[:, 1], xb_d[:])
            nc.sync.dma_start(w3_sb[:], w3_d[:])

            # stage-1 output: fp8 pair layout [ki, img, hp, wp], zero-padded
            s1p = pp.tile([128, 2, 2, 30, 32], fp8, tag="s1p", name="s1p")
            nc.vector.memset(s1p[:], 0.0)
            # stage-2 output: fp8 pair layout [ki, fb, col], fb = img*2+hb
            s2f = pp.tile([128, 2, 4, 400], fp8, tag="s2f", name="s2f")
            out_sb = pp.tile([128, 8, F], i8, tag="o", name="o")

            # per-channel scale/bias column views
            a1 = [vec_sb[:, m:m + 1] for m in range(2)]
            b1 = [vec_sb[:, 2 + m:3 + m] for m in range(2)]
            a2 = [vec_sb[:, 4 + m:5 + m] for m in range(2)]
            b2 = [vec_sb[:, 6 + m:7 + m] for m in range(2)]
            a3 = [vec_sb[:, 8 + m:9 + m] for m in range(8)]
            b3c = [vec_sb[:, 16 + k:17 + k] for k in range(8)]

            # ---- PE warm-up: dummy matmuls during the DMA window ----
            wps = psp.tile([128, 2, 512], f32, tag="ps", bufs=2, name="wps")
            for _ in range(N_WARMUP):
                nc.tensor.matmul(wps[:, 0, 0:128], warm[:], warm[:],
                                 start=True, stop=True)

            ps1, ps2, ps3 = {}, {}, {}

            # img0 x-chunks arrive as int8; fold beta3 and widen to f16
            def xfold(kt):
                nc.vector.tensor_scalar(xb_sb[:, 0, kt], x8_sb[:, kt],
                                        b3c[kt], None, Alu.add)

            def s1_mms_kt_interleaved(img):
                for m in range(2):
                    ps1[(m, img)] = psp.tile([128, 2, 512], f32, tag="ps",
                                             bufs=2, name=f"ps1_{m}{img}")
                for kt in range(8):
                    if kt == 0:
                        xfold(0)
                        xfold(1)
                    elif kt < 7:
                        xfold(kt + 1)
                    for m in range(2):
                        lhsT = w1_sb[:, kt * 2 + m]
                        for hb in range(2):
                            rhs = xb_sb[:, img, kt, hb * FB:(hb + 1) * FB]
                            nc.tensor.matmul(ps1[(m, img)][:, hb, 0:FB], lhsT,
                                             rhs, start=(kt == 0), stop=(kt == 7))

            def s1_mms_m_seq(img):
                for m in range(2):
                    ps1[(m, img)] = psp.tile([128, 2, 512], f32, tag="ps",
                                             bufs=2, name=f"ps1_{m}{img}")
                for m in range(2):
                    for kt in range(8):
                        lhsT = w1_sb[:, kt * 2 + m]
                        for hb in range(2):
                            rhs = xb_sb[:, img, kt, hb * FB:(hb + 1) * FB]
                            nc.tensor.matmul(ps1[(m, img)][:, hb, 0:FB], lhsT,
                                             rhs, start=(kt == 0), stop=(kt == 7))
                    if m == 0:
                        s1_epi_act(0, img)

            def s1_epi_act(m, img):
                nc.scalar.activation(s1p[:, m, img, 1:29, 1:29],
                                     ps1[(m, img)][:, :, 0:FB],
                                     Act.Relu, bias=b1[m], scale=a1[m])

            def s1_epi_dve(m, img):
                t = sp.tile([128, HW1], f32, tag="t1", name=f"t1_{m}{img}")
                nc.vector.tensor_scalar(t[:], ps1[(m, img)][:, :, 0:FB],
                                        a1[m], b1[m], Alu.mult, Alu.add)
                nc.vector.tensor_scalar(s1p[:, m, img, 1:29, 1:29], t[:],
                                        0.0, None, Alu.max)

            def s2_m(m, img):
                ps2[(m, img)] = psp.tile([128, 2, 512], f32, tag="ps",
                                         bufs=2, name=f"ps2_{m}{img}")

            def s2_taps(m, img, taps):
                for tap in taps:
                    dy, dx = tap // 3, tap % 3
                    lhsT = w2_sb[:, tap * 2 + m]
                    for hb in range(2):
                        h0 = hb * 14
                        rhs = s1p[:, :, img, h0 + dy:h0 + dy + 14, dx:dx + 28]
                        nc.tensor.matmul(
                            ps2[(m, img)][:, hb, 0:FB], lhsT, rhs,
                            start=(tap == 0), stop=(tap == 8), perf_mode=DR)

            def s2_epi(m, img):
                nc.scalar.activation(s2f[:, m, 2 * img:2 * img + 2, 0:FB],
                                     ps2[(m, img)][:, :, 0:FB],
                                     Act.Relu, bias=b2[m], scale=a2[m])

            def s3_mm(m, img):
                tag = "ps" if (img == 1 and m % 2 == 0) or (m, img) == (2, 0) else "ps3"
                p = psp.tile([128, 2, 512], f32, tag=tag, bufs=2,
                             name=f"ps3_{m}{img}")
                ps3[(m, img)] = p
                for hb in range(2):
                    fb = img * 2 + hb
                    nc.tensor.matmul(p[:, hb, 0:FB], w3_sb[:, m],
                                     s2f[:, :, fb, 0:FB],
                                     start=True, stop=True, perf_mode=DR)

            def s3_epi(m, img):
                osl = out_sb[:, m, img * HW1:(img + 1) * HW1]
                xsl = xb_sb[:, img, m]
                if (m, img) in S3_GP_UNITS:
                    t = sp.tile([128, HW1], f16, tag="t3", name=f"t3_{m}{img}")
                    t2 = sp.tile([128, HW1], f16, tag="t4", name=f"t4_{m}{img}")
                    nc.scalar.activation(t[:], ps3[(m, img)][:, :, 0:FB],
                                         Act.Copy, bias=0.0, scale=a3[m])
                    nc.gpsimd.tensor_tensor(t2[:], t[:], xsl, Alu.add)
                    nc.vector.tensor_scalar(osl, t2[:], 0.0, None, Alu.add)
                else:
                    nc.vector.scalar_tensor_tensor(
                        osl, ps3[(m, img)][:, :, 0:FB], a3[m], xsl,
                        Alu.mult, Alu.add)
                nc.sync.dma_start(out_d[:, m, img * HW1:(img + 1) * HW1], osl)

            def s3_unit(m, img):
                s3_mm(m, img)
                s3_epi(m, img)

            # ---- pipelined schedule ----
            # img0 stage 1: kt-interleaved (rides the DMA arrivals)
            s1_mms_kt_interleaved(0)
            s1_epi_act(0, 0)
            s1_epi_act(1, 0)
            # img0 stage 2: m1 first so its psum frees early for s1(img1)
            s2_m(1, 0)
            s2_taps(1, 0, range(9))
            s2_epi(1, 0)
            s2_m(0, 0)
            s2_taps(0, 0, range(9))
            s2_epi(0, 0)
            # img1 stage 1: m-sequential (m0 chain, epi, m1 chain)
            s1_mms_m_seq(1)
            s1_epi_act(1, 1)
            # img0 stage 3 head starts while img1 epilogues settle
            s3_unit(0, 0)
            s3_unit(1, 0)
            s3_unit(2, 0)
            # img1 stage 2 with img0 stage-3 units interleaved
            s2_m(0, 1)
            s2_taps(0, 1, range(0, 4))
            s3_unit(3, 0)
            s2_taps(0, 1, range(4, 7))
            s3_unit(4, 0)
            s2_taps(0, 1, range(7, 9))
            s2_epi(0, 1)
            s3_unit(5, 0)
            s2_m(1, 1)
            s2_taps(1, 1, range(0, 4))
            s3_unit(6, 0)
            s2_taps(1, 1, range(4, 7))
            s3_unit(7, 0)
            s2_taps(1, 1, range(7, 9))
            s2_epi(1, 1)
            # img1 stage 3
            for m in range(8):
                s3_unit(m, 1)

    nc.compile()
    return nc


def _get_nc():
    if "nc" not in _CACHE:
        _CACHE["nc"] = _build()
    return _CACHE["nc"]


def _pack_inputs(inputs):
    """Host-side: effective weights, bias folds, per-core shards, casts."""
    f32 = np.float32
    f64 = np.float64
    x = np.asarray(inputs["x"])

    def eff(w2, s):
        return (np.asarray(w2, dtype=f32) *
                np.exp2(np.asarray(s).astype(f32))).astype(f32)

    w1e = eff(inputs["w2_1"], inputs["s1"])[:, :, 0, 0]          # [O=256, I=1024]
    w1 = np.ascontiguousarray(
        w1e.T.reshape(8, 128, 2, 128).transpose(1, 0, 2, 3)
        .reshape(128, 16, 128)).astype(F16)
    w2e = eff(inputs["w2_2"], inputs["s2"])                      # [O, I, 3, 3]
    taps = np.stack([w2e[:, :, dy, dx].T
                     for dy in range(3) for dx in range(3)])     # [9, I, O]
    w2 = np.ascontiguousarray(
        taps.reshape(9, 2, 128, 2, 128)
        .transpose(2, 0, 3, 1, 4)
        .reshape(128, 18, 2, 128)).astype(FP8)
    w3e = eff(inputs["w2_3"], inputs["s3"])[:, :, 0, 0]          # [O=1024, I=256]
    w3 = np.ascontiguousarray(
        w3e.T.reshape(2, 128, 8, 128)
        .transpose(1, 2, 0, 3)).astype(FP8)

    scl = np.exp2(f32(-12.0))
    b3p = (np.asarray(inputs["beta3"], dtype=f32) *
           np.exp2(np.asarray(inputs["q3"]).astype(f32)))        # [1024]
    a1f = np.asarray(inputs["alpha1"], dtype=f32) * scl
    b1f = (np.asarray(inputs["beta1"], dtype=f32) *
           np.exp2(np.asarray(inputs["q1"]).astype(f32)))
    # stage-1 bias correction for the beta3 folded into xb
    corr = w1e.astype(f64) @ b3p.astype(f64)                     # [256]
    b1c = (b1f.astype(f64) - a1f.astype(f64) * corr).astype(f32)

    vec = np.zeros((128, 24), dtype=f32)
    for m in range(2):
        sl = slice(m * 128, (m + 1) * 128)
        vec[:, m] = a1f[sl]
        vec[:, 2 + m] = b1c[sl]
        vec[:, 4 + m] = np.asarray(inputs["alpha2"], dtype=f32)[sl] * scl
        vec[:, 6 + m] = (np.asarray(inputs["beta2"], dtype=f32)[sl] *
                         np.exp2(np.asarray(inputs["q2"]).astype(f32)[sl]))
    for m in range(8):
        sl = slice(m * 128, (m + 1) * 128)
        vec[:, 8 + m] = np.asarray(inputs["alpha3"], dtype=f32)[sl] * scl

    for k in range(8):
        vec[:, 16 + k] = b3p[k * 128:(k + 1) * 128]

    xb = x.astype(f32) + b3p[None, :, None, None]                # [16,1024,28,28]
    in_maps = []
    for c in range(N_CORES):
        csl = slice(c * N_PER_CORE, (c + 1) * N_PER_CORE)
        x8c = np.ascontiguousarray(
            x[csl][0].reshape(8, 128, HW1)
            .transpose(1, 0, 2)).astype(np.int8)                 # [128,8,784]
        xbc = np.ascontiguousarray(
            xb[csl][1].reshape(8, 128, HW1)
            .transpose(1, 0, 2)).astype(F16)                     # [128,8,784]
        in_maps.append({"x8": x8c, "xb": xbc, "w1": w1, "w2": w2, "w3": w3,
                        "vec": vec})
    return in_maps


def _assemble(results):
    outs = []
    for c in range(N_CORES):
        o = results[c]["out"]                                    # [128,8,1568] i8
        o = np.maximum(o, 0).astype(np.float32)                  # final relu
        # [128p, 8kt, 2img*784] -> [2, 1024, 28, 28]
        o = o.reshape(128, 8, 2, 28, 28).transpose(2, 1, 0, 3, 4).reshape(
            N_PER_CORE, 1024, 28, 28)
        outs.append(o)
    return np.concatenate(outs, axis=0)


def _run(inputs, trace=False, **kwargs):
    from concourse.bass_utils import run_bass_kernel_spmd
    nc = _get_nc()
    in_maps = _pack_inputs(inputs)
    res = run_bass_kernel_spmd(nc, in_maps, list(range(N_CORES)),
                               trace=trace, **kwargs)
    return _assemble(res.results), res


def kernel(**inputs):
    out, _ = _run(inputs)
    return out


# revision 15
# speedup vs baseline: 1.1944x; 1.1944x over previous
"""Bottleneck residual block (1x1 -> 3x3 -> 1x1 conv + BN + residual) on 8 NeuronCores.

Strategy: data-parallel over batch (16 images -> 2 per core), software
pipelined at image granularity: img0 flows stage1->stage2->stage3 while
img1's stage1/2 matmuls fill the PE, so img0's stage-3 epilogues hide
under img1 compute.

  - one upload xb = f16(x + beta3*2^q3): stage 1 consumes it with a
    host-corrected bias (so beta3 costs nothing), stage 3 adds it as the
    pre-biased residual.
  - stage 1 runs f16 (K=1024), stages 2+3 fp8e4m3 DoubleRow (K=256/pass).
  - stages 1/2 epilogue: one ACT op (or a 2-op DVE chain) per (m, img):
    Relu(psum*a+b) -> fp8 direct.
  - stage 3 epilogue: one DVE scalar_tensor_tensor per (m, img):
    (psum*a3 + xb) -> int8 with RNE + saturation (exact round + clip at
    127); final relu done host-side on the int8 results (max(out,0)).
    A couple of tail units run ACT-drain + GpSimd f16 add + DVE convert
    to spread the load.
Intermediate BN rounding (round-to-nearest of bn1/bn2) is skipped: the
error enters the next conv scaled by alpha*2^-12 (~1e-5); end-to-end rel
err is ~1.6e-3, far under the 2e-2 gate (validated against reference).

DMA triggers are split across the Sync and ACT HWDGE queues (trigger
issue costs ~0.65us each, which otherwise paces the input stream); a
dummy ACT op preloads the activation table; dummy warm-up matmuls ramp
the PE HAM clock to 2.4GHz during the initial DMA window.

Shapes hardcoded for N=16, Cin=Cout=1024, width=256, H=W=28.
"""

import numpy as np
import ml_dtypes

BF16 = ml_dtypes.bfloat16
FP8 = ml_dtypes.float8_e4m3
F16 = np.float16

N_CORES = 8
N_PER_CORE = 2          # images per core
HW1 = 28 * 28           # 784 spatial positions per image
F = N_PER_CORE * HW1    # 1568 free-dim elements per core
FB = 392                # matmul free-dim block (14 rows of 28)

N_WARMUP = 22           # dummy matmuls to ramp the PE clock

# stage-3 units using the ACT+GpSimd+DVE-convert spread (img1 tail only)
S3_GP_UNITS = {(3, 0), (5, 0)}

_CACHE = {}


def _build():
    import concourse.bacc as bacc
    import concourse.mybir as mybir
    import concourse.tile as tile

    dt = mybir.dt
    f32, f16, i8, fp8 = dt.float32, dt.float16, dt.int8, dt.float8e4
    Alu = mybir.AluOpType
    Act = mybir.ActivationFunctionType
    DR = mybir.MatmulPerfMode.DoubleRow

    nc = bacc.Bacc("TRN2", target_bir_lowering=False, debug=False,
                   num_devices=N_CORES, enable_partition_id=False)

    x8_d = nc.dram_tensor("x8", [128, 8, HW1], i8, kind="ExternalInput")
    xb_d = nc.dram_tensor("xb", [128, 8, HW1], f16, kind="ExternalInput")
    w1_d = nc.dram_tensor("w1", [128, 16, 128], f16, kind="ExternalInput")
    w2_d = nc.dram_tensor("w2", [128, 18, 2, 128], fp8, kind="ExternalInput")
    w3_d = nc.dram_tensor("w3", [128, 8, 2, 128], fp8, kind="ExternalInput")
    vec_d = nc.dram_tensor("vec", [128, 24], f32, kind="ExternalInput")
    out_d = nc.dram_tensor("out", [128, 8, F], i8, kind="ExternalOutput")

    with tile.TileContext(nc) as tc:
        with (
            tc.tile_pool(name="persist", bufs=1) as pp,
            tc.tile_pool(name="stage", bufs=4) as sp,
            tc.tile_pool(name="psum", bufs=1, space="PSUM") as psp,
        ):
            # ---- persistent SBUF tensors ----
            warm = pp.tile([128, 128], f16, tag="warm", name="warm")
            nc.vector.memset(warm[:], 0.0)
            # preload the ACT function table before it's needed
            twu = sp.tile([128, 1], f32, tag="twu", name="twu")
            nc.scalar.activation(twu[:], warm[:, 0:1], Act.Relu)

            xb_sb = pp.tile([128, 2, 8, HW1], f16, tag="xb", name="xb")
            x8_sb = pp.tile([128, 8, HW1], i8, tag="x8", name="x8")
            w1_sb = pp.tile([128, 16, 128], f16, tag="w1", name="w1")
            vec_sb = pp.tile([128, 24], f32, tag="vec", name="vec")
            w2_sb = pp.tile([128, 18, 2, 128], fp8, tag="w2", name="w2")
            w3_sb = pp.tile([128, 8, 2, 128], fp8, tag="w3", name="w3")

            # input DMA: one queue, strictly in stage-1 consumption order
            # (concurrent transfers steal bandwidth from the critical chunk);
            # only the tiny vec rides the ACT queue.
            nc.sync.dma_start(w1_sb[:, 0:4], w1_d[:, 0:4])
            nc.sync.dma_start(x8_sb[:, 0:2], x8_d[:, 0:2])
            nc.scalar.dma_start(vec_sb[:], vec_d[:])
            nc.sync.dma_start(w1_sb[:, 4:10], w1_d[:, 4:10])
            nc.sync.dma_start(x8_sb[:, 2:5], x8_d[:, 2:5])
            nc.sync.dma_start(w1_sb[:, 10:16], w1_d[:, 10:16])
            nc.sync.dma_start(x8_sb[:, 5:8], x8_d[:, 5:8])
            nc.sync.dma_start(w2_sb[:], w2_d[:])
            nc.sync.dma_start(xb_sb[:, 1], xb_d[:])
            nc.sync.dma_start(w3_sb[:], w3_d[:])

            # stage-1 output: fp8 pair layout [ki, img, hp, wp], zero-padded
            s1p = pp.tile([128, 2, 2, 30, 32], fp8, tag="s1p", name="s1p")
            nc.gpsimd.memset(s1p[:], 0.0)
            # stage-2 output: fp8 pair layout [ki, fb, col], fb = img*2+hb
            s2f = pp.tile([128, 2, 4, 400], fp8, tag="s2f", name="s2f")
            out_sb = pp.tile([128, 8, F], i8, tag="o", name="o")

            # per-channel scale/bias column views
            a1 = [vec_sb[:, m:m + 1] for m in range(2)]
            b1 = [vec_sb[:, 2 + m:3 + m] for m in range(2)]
            a2 = [vec_sb[:, 4 + m:5 + m] for m in range(2)]
            b2 = [vec_sb[:, 6 + m:7 + m] for m in range(2)]
            a3 = [vec_sb[:, 8 + m:9 + m] for m in range(8)]
            b3c = [vec_sb[:, 16 + k:17 + k] for k in range(8)]

            # ---- PE warm-up: dummy matmuls during the DMA window ----
            wps = psp.tile([128, 2, 512], f32, tag="ps", bufs=2, name="wps")
            for _ in range(N_WARMUP):
                nc.tensor.matmul(wps[:, 0, 0:128], warm[:], warm[:],
                                 start=True, stop=True)

            ps1, ps2, ps3 = {}, {}, {}

            # img0 x-chunks arrive as int8; fold beta3 and widen to f16
            def xfold(kt):
                nc.vector.tensor_scalar(xb_sb[:, 0, kt], x8_sb[:, kt],
                                        b3c[kt], None, Alu.add)

            def s1_mms_kt_interleaved(img):
                for m in range(2):
                    ps1[(m, img)] = psp.tile([128, 2, 512], f32, tag="ps",
                                             bufs=2, name=f"ps1_{m}{img}")
                for kt in range(8):
                    if kt == 0:
                        xfold(0)
                        xfold(1)
                    elif kt < 7:
                        xfold(kt + 1)
                    for m in range(2):
                        lhsT = w1_sb[:, kt * 2 + m]
                        for hb in range(2):
                            rhs = xb_sb[:, img, kt, hb * FB:(hb + 1) * FB]
                            nc.tensor.matmul(ps1[(m, img)][:, hb, 0:FB], lhsT,
                                             rhs, start=(kt == 0), stop=(kt == 7))

            def s1_mms_m_seq(img):
                for m in range(2):
                    ps1[(m, img)] = psp.tile([128, 2, 512], f32, tag="ps",
                                             bufs=2, name=f"ps1_{m}{img}")
                for m in range(2):
                    for kt in range(8):
                        lhsT = w1_sb[:, kt * 2 + m]
                        for hb in range(2):
                            rhs = xb_sb[:, img, kt, hb * FB:(hb + 1) * FB]
                            nc.tensor.matmul(ps1[(m, img)][:, hb, 0:FB], lhsT,
                                             rhs, start=(kt == 0), stop=(kt == 7))
                    if m == 0:
                        s1_epi_act(0, img)

            def s1_epi_act(m, img):
                nc.scalar.activation(s1p[:, m, img, 1:29, 1:29],
                                     ps1[(m, img)][:, :, 0:FB],
                                     Act.Relu, bias=b1[m], scale=a1[m])

            def s1_epi_dve(m, img):
                t = sp.tile([128, HW1], f32, tag="t1", name=f"t1_{m}{img}")
                nc.vector.tensor_scalar(t[:], ps1[(m, img)][:, :, 0:FB],
                                        a1[m], b1[m], Alu.mult, Alu.add)
                nc.vector.tensor_scalar(s1p[:, m, img, 1:29, 1:29], t[:],
                                        0.0, None, Alu.max)

            def s2_m(m, img):
                ps2[(m, img)] = psp.tile([128, 2, 512], f32, tag="ps",
                                         bufs=2, name=f"ps2_{m}{img}")

            def s2_taps(m, img, taps):
                for tap in taps:
                    dy, dx = tap // 3, tap % 3
                    lhsT = w2_sb[:, tap * 2 + m]
                    for hb in range(2):
                        h0 = hb * 14
                        rhs = s1p[:, :, img, h0 + dy:h0 + dy + 14, dx:dx + 28]
                        nc.tensor.matmul(
                            ps2[(m, img)][:, hb, 0:FB], lhsT, rhs,
                            start=(tap == 0), stop=(tap == 8), perf_mode=DR)

            def s2_epi(m, img):
                nc.scalar.activation(s2f[:, m, 2 * img:2 * img + 2, 0:FB],
                                     ps2[(m, img)][:, :, 0:FB],
                                     Act.Relu, bias=b2[m], scale=a2[m])

            def s3_mm(m, img):
                tag = "ps" if (img == 1 and m % 2 == 0) or (m, img) == (2, 0) else "ps3"
                p = psp.tile([128, 2, 512], f32, tag=tag, bufs=2,
                             name=f"ps3_{m}{img}")
                ps3[(m, img)] = p
                for hb in range(2):
                    fb = img * 2 + hb
                    nc.tensor.matmul(p[:, hb, 0:FB], w3_sb[:, m],
                                     s2f[:, :, fb, 0:FB],
                                     start=True, stop=True, perf_mode=DR)

            def s3_epi(m, img):
                osl = out_sb[:, m, img * HW1:(img + 1) * HW1]
                xsl = xb_sb[:, img, m]
                if (m, img) in S3_GP_UNITS:
                    t = sp.tile([128, HW1], f16, tag="t3", name=f"t3_{m}{img}")
                    t2 = sp.tile([128, HW1], f16, tag="t4", name=f"t4_{m}{img}")
                    nc.scalar.activation(t[:], ps3[(m, img)][:, :, 0:FB],
                                         Act.Copy, bias=0.0, scale=a3[m])
                    nc.gpsimd.tensor_tensor(t2[:], t[:], xsl, Alu.add)
                    nc.vector.tensor_scalar(osl, t2[:], 0.0, None, Alu.add)
                else:
                    nc.vector.scalar_tensor_tensor(
                        osl, ps3[(m, img)][:, :, 0:FB], a3[m], xsl,
                        Alu.mult, Alu.add)
                nc.sync.dma_start(out_d[:, m, img * HW1:(img + 1) * HW1], osl)

            def s3_unit(m, img):
                s3_mm(m, img)
                s3_epi(m, img)

            # ---- pipelined schedule ----
            # img0 stage 1: kt-interleaved (rides the DMA arrivals)
            s1_mms_kt_interleaved(0)
            s1_epi_act(0, 0)
            s1_epi_act(1, 0)
            # img0 stage 2: m1 first so its psum frees early for s1(img1)
            s2_m(1, 0)
            s2_taps(1, 0, range(9))
            s2_epi(1, 0)
            s2_m(0, 0)
            s2_taps(0, 0, range(9))
            s2_epi(0, 0)
            # img1 stage 1: m-sequential (m0 chain, epi, m1 chain)
            s1_mms_m_seq(1)
            s1_epi_act(1, 1)
            # img0 stage 3 head starts while img1 epilogues settle
            s3_unit(0, 0)
            s3_unit(1, 0)
            s3_unit(2, 0)
            # img1 stage 2 with img0 stage-3 units interleaved
            s2_m(0, 1)
            s2_taps(0, 1, range(0, 4))
            s3_unit(3, 0)
            s2_taps(0, 1, range(4, 7))
            s3_unit(4, 0)
            s2_taps(0, 1, range(7, 9))
            s2_epi(0, 1)
            s3_unit(5, 0)
            s2_m(1, 1)
            s2_taps(1, 1, range(0, 4))
            s3_unit(6, 0)
            s2_taps(1, 1, range(4, 7))
            s3_unit(7, 0)
            s2_taps(1, 1, range(7, 9))
            s2_epi(1, 1)
            # img1 stage 3
            for m in range(8):
                s3_unit(m, 1)

    nc.compile()
    return nc


def _get_nc():
    if "nc" not in _CACHE:
        _CACHE["nc"] = _build()
    return _CACHE["nc"]


def _pack_inputs(inputs):
    """Host-side: effective weights, bias folds, per-core shards, casts."""
    f32 = np.float32
    f64 = np.float64
    x = np.asarray(inputs["x"])

    def eff(w2, s):
        return (np.asarray(w2, dtype=f32) *
                np.exp2(np.asarray(s).astype(f32))).astype(f32)

    w1e = eff(inputs["w2_1"], inputs["s1"])[:, :, 0, 0]          # [O=256, I=1024]
    w1 = np.ascontiguousarray(
        w1e.T.reshape(8, 128, 2, 128).transpose(1, 0, 2, 3)
        .reshape(128, 16, 128)).astype(F16)
    w2e = eff(inputs["w2_2"], inputs["s2"])                      # [O, I, 3, 3]
    taps = np.stack([w2e[:, :, dy, dx].T
                     for dy in range(3) for dx in range(3)])     # [9, I, O]
    w2 = np.ascontiguousarray(
        taps.reshape(9, 2, 128, 2, 128)
        .transpose(2, 0, 3, 1, 4)
        .reshape(128, 18, 2, 128)).astype(FP8)
    w3e = eff(inputs["w2_3"], inputs["s3"])[:, :, 0, 0]          # [O=1024, I=256]
    w3 = np.ascontiguousarray(
        w3e.T.reshape(2, 128, 8, 128)
        .transpose(1, 2, 0, 3)).astype(FP8)

    scl = np.exp2(f32(-12.0))
    b3p = (np.asarray(inputs["beta3"], dtype=f32) *
           np.exp2(np.asarray(inputs["q3"]).astype(f32)))        # [1024]
    a1f = np.asarray(inputs["alpha1"], dtype=f32) * scl
    b1f = (np.asarray(inputs["beta1"], dtype=f32) *
           np.exp2(np.asarray(inputs["q1"]).astype(f32)))
    # stage-1 bias correction for the beta3 folded into xb
    corr = w1e.astype(f64) @ b3p.astype(f64)                     # [256]
    b1c = (b1f.astype(f64) - a1f.astype(f64) * corr).astype(f32)

    vec = np.zeros((128, 24), dtype=f32)
    for m in range(2):
        sl = slice(m * 128, (m + 1) * 128)
        vec[:, m] = a1f[sl]
        vec[:, 2 + m] = b1c[sl]
        vec[:, 4 + m] = np.asarray(inputs["alpha2"], dtype=f32)[sl] * scl
        vec[:, 6 + m] = (np.asarray(inputs["beta2"], dtype=f32)[sl] *
                         np.exp2(np.asarray(inputs["q2"]).astype(f32)[sl]))
    for m in range(8):
        sl = slice(m * 128, (m + 1) * 128)
        vec[:, 8 + m] = np.asarray(inputs["alpha3"], dtype=f32)[sl] * scl

    for k in range(8):
        vec[:, 16 + k] = b3p[k * 128:(k + 1) * 128]

    xb = x.astype(f32) + b3p[None, :, None, None]                # [16,1024,28,28]
    in_maps = []
    for c in range(N_CORES):
        csl = slice(c * N_PER_CORE, (c + 1) * N_PER_CORE)
        x8c = np.ascontiguousarray(
            x[csl][0].reshape(8, 128, HW1)
            .transpose(1, 0, 2)).astype(np.int8)                 # [128,8,784]
        xbc = np.ascontiguousarray(
            xb[csl][1].reshape(8, 128, HW1)
            .transpose(1, 0, 2)).astype(F16)                     # [128,8,784]
        in_maps.append({"x8": x8c, "xb": xbc, "w1": w1, "w2": w2, "w3": w3,
                        "vec": vec})
    return in_maps


def _assemble(results):
    outs = []
    for c in range(N_CORES):
        o = results[c]["out"]                                    # [128,8,1568] i8
        o = np.maximum(o, 0).astype(np.float32)                  # final relu
        # [128p, 8kt, 2img*784] -> [2, 1024, 28, 28]
        o = o.reshape(128, 8, 2, 28, 28).transpose(2, 1, 0, 3, 4).reshape(
            N_PER_CORE, 1024, 28, 28)
        outs.append(o)
    return np.concatenate(outs, axis=0)


def _run(inputs, trace=False, **kwargs):
    from concourse.bass_utils import run_bass_kernel_spmd
    nc = _get_nc()
    in_maps = _pack_inputs(inputs)
    res = run_bass_kernel_spmd(nc, in_maps, list(range(N_CORES)),
                               trace=trace, **kwargs)
    return _assemble(res.results), res


def kernel(**inputs):
    out, _ = _run(inputs)
    return out


# revision 16
# speedup vs baseline: 1.2109x; 1.0139x over previous
"""Bottleneck residual block (1x1 -> 3x3 -> 1x1 conv + BN + residual) on 8 NeuronCores.

Strategy: data-parallel over batch (16 images -> 2 per core), software
pipelined at image granularity: img0 flows stage1->stage2->stage3 while
img1's stage1/2 matmuls fill the PE, so img0's stage-3 epilogues hide
under img1 compute.

  - one upload xb = f16(x + beta3*2^q3): stage 1 consumes it with a
    host-corrected bias (so beta3 costs nothing), stage 3 adds it as the
    pre-biased residual.
  - stage 1 runs f16 (K=1024), stages 2+3 fp8e4m3 DoubleRow (K=256/pass).
  - stages 1/2 epilogue: one ACT op (or a 2-op DVE chain) per (m, img):
    Relu(psum*a+b) -> fp8 direct.
  - stage 3 epilogue: one DVE scalar_tensor_tensor per (m, img):
    (psum*a3 + xb) -> int8 with RNE + saturation (exact round + clip at
    127); final relu done host-side on the int8 results (max(out,0)).
    A couple of tail units run ACT-drain + GpSimd f16 add + DVE convert
    to spread the load.
Intermediate BN rounding (round-to-nearest of bn1/bn2) is skipped: the
error enters the next conv scaled by alpha*2^-12 (~1e-5); end-to-end rel
err is ~1.6e-3, far under the 2e-2 gate (validated against reference).

DMA triggers are split across the Sync and ACT HWDGE queues (trigger
issue costs ~0.65us each, which otherwise paces the input stream); a
dummy ACT op preloads the activation table; dummy warm-up matmuls ramp
the PE HAM clock to 2.4GHz during the initial DMA window.

Shapes hardcoded for N=16, Cin=Cout=1024, width=256, H=W=28.
"""

import numpy as np
import ml_dtypes

BF16 = ml_dtypes.bfloat16
FP8 = ml_dtypes.float8_e4m3
F16 = np.float16

N_CORES = 8
N_PER_CORE = 2          # images per core
HW1 = 28 * 28           # 784 spatial positions per image
F = N_PER_CORE * HW1    # 1568 free-dim elements per core
FB = 392                # matmul free-dim block (14 rows of 28)

N_WARMUP = 32           # dummy matmuls to ramp the PE clock

# stage-3 units using the ACT+GpSimd+DVE-convert spread (img1 tail only)
S3_GP_UNITS = {(3, 0), (5, 0)}

_CACHE = {}


def _build():
    import concourse.bacc as bacc
    import concourse.mybir as mybir
    import concourse.tile as tile

    dt = mybir.dt
    f32, f16, i8, fp8 = dt.float32, dt.float16, dt.int8, dt.float8e4
    Alu = mybir.AluOpType
    Act = mybir.ActivationFunctionType
    DR = mybir.MatmulPerfMode.DoubleRow

    nc = bacc.Bacc("TRN2", target_bir_lowering=False, debug=False,
                   num_devices=N_CORES, enable_partition_id=False)

    x8_d = nc.dram_tensor("x8", [128, 8, HW1], i8, kind="ExternalInput")
    xb_d = nc.dram_tensor("xb", [128, 8, HW1], f16, kind="ExternalInput")
    w1_d = nc.dram_tensor("w1", [128, 16, 128], f16, kind="ExternalInput")
    w2_d = nc.dram_tensor("w2", [128, 18, 2, 128], fp8, kind="ExternalInput")
    w3_d = nc.dram_tensor("w3", [128, 8, 2, 128], fp8, kind="ExternalInput")
    vec_d = nc.dram_tensor("vec", [128, 24], f32, kind="ExternalInput")
    out_d = nc.dram_tensor("out", [128, 8, F], i8, kind="ExternalOutput")

    with tile.TileContext(nc) as tc:
        with (
            tc.tile_pool(name="persist", bufs=1) as pp,
            tc.tile_pool(name="stage", bufs=4) as sp,
            tc.tile_pool(name="psum", bufs=1, space="PSUM") as psp,
        ):
            # ---- persistent SBUF tensors ----
            warm = pp.tile([128, 128], f16, tag="warm", name="warm")
            nc.vector.memset(warm[:], 0.0)
            # preload the ACT function table before it's needed
            twu = sp.tile([128, 1], f32, tag="twu", name="twu")
            nc.scalar.activation(twu[:], warm[:, 0:1], Act.Relu)

            xb_sb = pp.tile([128, 2, 8, HW1], f16, tag="xb", name="xb")
            x8_sb = pp.tile([128, 8, HW1], i8, tag="x8", name="x8")
            w1_sb = pp.tile([128, 16, 128], f16, tag="w1", name="w1")
            vec_sb = pp.tile([128, 24], f32, tag="vec", name="vec")
            w2_sb = pp.tile([128, 18, 2, 128], fp8, tag="w2", name="w2")
            w3_sb = pp.tile([128, 8, 2, 128], fp8, tag="w3", name="w3")

            # input DMA: one queue, strictly in stage-1 consumption order
            # (concurrent transfers steal bandwidth from the critical chunk);
            # only the tiny vec rides the ACT queue.
            nc.sync.dma_start(w1_sb[:, 0:4], w1_d[:, 0:4])
            nc.sync.dma_start(x8_sb[:, 0:2], x8_d[:, 0:2])
            nc.scalar.dma_start(vec_sb[:], vec_d[:])
            nc.sync.dma_start(w1_sb[:, 4:10], w1_d[:, 4:10])
            nc.sync.dma_start(x8_sb[:, 2:5], x8_d[:, 2:5])
            nc.sync.dma_start(w1_sb[:, 10:16], w1_d[:, 10:16])
            nc.sync.dma_start(x8_sb[:, 5:8], x8_d[:, 5:8])
            nc.sync.dma_start(w2_sb[:], w2_d[:])
            nc.sync.dma_start(xb_sb[:, 1], xb_d[:])
            nc.sync.dma_start(w3_sb[:], w3_d[:])

            # stage-1 output: fp8 pair layout [ki, img, hp, wp], zero-padded
            s1p = pp.tile([128, 2, 2, 30, 32], fp8, tag="s1p", name="s1p")
            nc.gpsimd.memset(s1p[:], 0.0)
            # stage-2 output: fp8 pair layout [ki, fb, col], fb = img*2+hb
            s2f = pp.tile([128, 2, 4, 400], fp8, tag="s2f", name="s2f")
            out_sb = pp.tile([128, 8, F], i8, tag="o", name="o")

            # per-channel scale/bias column views
            a1 = [vec_sb[:, m:m + 1] for m in range(2)]
            b1 = [vec_sb[:, 2 + m:3 + m] for m in range(2)]
            a2 = [vec_sb[:, 4 + m:5 + m] for m in range(2)]
            b2 = [vec_sb[:, 6 + m:7 + m] for m in range(2)]
            a3 = [vec_sb[:, 8 + m:9 + m] for m in range(8)]
            b3c = [vec_sb[:, 16 + k:17 + k] for k in range(8)]

            # ---- PE warm-up: dummy matmuls during the DMA window ----
            wps = psp.tile([128, 2, 512], f32, tag="ps", bufs=2, name="wps")
            for _ in range(N_WARMUP):
                nc.tensor.matmul(wps[:, 0, 0:128], warm[:], warm[:],
                                 start=True, stop=True)

            ps1, ps2, ps3 = {}, {}, {}

            # img0 x-chunks arrive as int8; fold beta3 and widen to f16
            def xfold(kt):
                nc.vector.tensor_scalar(xb_sb[:, 0, kt], x8_sb[:, kt],
                                        b3c[kt], None, Alu.add)

            def s1_mms_kt_interleaved(img):
                for m in range(2):
                    ps1[(m, img)] = psp.tile([128, 2, 512], f32, tag="ps",
                                             bufs=2, name=f"ps1_{m}{img}")
                for kt in range(8):
                    if kt == 0:
                        xfold(0)
                        xfold(1)
                    elif kt < 7:
                        xfold(kt + 1)
                    for m in range(2):
                        lhsT = w1_sb[:, kt * 2 + m]
                        for hb in range(2):
                            rhs = xb_sb[:, img, kt, hb * FB:(hb + 1) * FB]
                            nc.tensor.matmul(ps1[(m, img)][:, hb, 0:FB], lhsT,
                                             rhs, start=(kt == 0), stop=(kt == 7))

            def s1_mms_m_seq(img):
                for m in range(2):
                    ps1[(m, img)] = psp.tile([128, 2, 512], f32, tag="ps",
                                             bufs=2, name=f"ps1_{m}{img}")
                for m in range(2):
                    for kt in range(8):
                        lhsT = w1_sb[:, kt * 2 + m]
                        for hb in range(2):
                            rhs = xb_sb[:, img, kt, hb * FB:(hb + 1) * FB]
                            nc.tensor.matmul(ps1[(m, img)][:, hb, 0:FB], lhsT,
                                             rhs, start=(kt == 0), stop=(kt == 7))
                    if m == 0:
                        s1_epi_act(0, img)

            def s1_epi_act(m, img):
                nc.scalar.activation(s1p[:, m, img, 1:29, 1:29],
                                     ps1[(m, img)][:, :, 0:FB],
                                     Act.Relu, bias=b1[m], scale=a1[m])

            def s1_epi_dve(m, img):
                t = sp.tile([128, HW1], f32, tag="t1", name=f"t1_{m}{img}")
                nc.vector.tensor_scalar(t[:], ps1[(m, img)][:, :, 0:FB],
                                        a1[m], b1[m], Alu.mult, Alu.add)
                nc.vector.tensor_scalar(s1p[:, m, img, 1:29, 1:29], t[:],
                                        0.0, None, Alu.max)

            def s2_m(m, img):
                ps2[(m, img)] = psp.tile([128, 2, 512], f32, tag="ps",
                                         bufs=2, name=f"ps2_{m}{img}")

            def s2_taps(m, img, taps):
                for tap in taps:
                    dy, dx = tap // 3, tap % 3
                    lhsT = w2_sb[:, tap * 2 + m]
                    for hb in range(2):
                        h0 = hb * 14
                        rhs = s1p[:, :, img, h0 + dy:h0 + dy + 14, dx:dx + 28]
                        nc.tensor.matmul(
                            ps2[(m, img)][:, hb, 0:FB], lhsT, rhs,
                            start=(tap == 0), stop=(tap == 8), perf_mode=DR)

            def s2_epi(m, img):
                nc.scalar.activation(s2f[:, m, 2 * img:2 * img + 2, 0:FB],
                                     ps2[(m, img)][:, :, 0:FB],
                                     Act.Relu, bias=b2[m], scale=a2[m])

            def s3_mm(m, img):
                tag = "ps" if (img == 1 and m % 2 == 0) or (m, img) == (2, 0) else "ps3"
                p = psp.tile([128, 2, 512], f32, tag=tag, bufs=2,
                             name=f"ps3_{m}{img}")
                ps3[(m, img)] = p
                for hb in range(2):
                    fb = img * 2 + hb
                    nc.tensor.matmul(p[:, hb, 0:FB], w3_sb[:, m],
                                     s2f[:, :, fb, 0:FB],
                                     start=True, stop=True, perf_mode=DR)

            def s3_epi(m, img):
                osl = out_sb[:, m, img * HW1:(img + 1) * HW1]
                xsl = xb_sb[:, img, m]
                if (m, img) in S3_GP_UNITS:
                    t = sp.tile([128, HW1], f16, tag="t3", name=f"t3_{m}{img}")
                    t2 = sp.tile([128, HW1], f16, tag="t4", name=f"t4_{m}{img}")
                    nc.scalar.activation(t[:], ps3[(m, img)][:, :, 0:FB],
                                         Act.Copy, bias=0.0, scale=a3[m])
                    nc.gpsimd.tensor_tensor(t2[:], t[:], xsl, Alu.add)
                    nc.vector.tensor_scalar(osl, t2[:], 0.0, None, Alu.add)
                else:
                    nc.vector.scalar_tensor_tensor(
                        osl, ps3[(m, img)][:, :, 0:FB], a3[m], xsl,
                        Alu.mult, Alu.add)
                nc.sync.dma_start(out_d[:, m, img * HW1:(img + 1) * HW1], osl)

            def s3_unit(m, img):
                s3_mm(m, img)
                s3_epi(m, img)

            # ---- pipelined schedule ----
            # img0 stage 1: kt-interleaved (rides the DMA arrivals)
            s1_mms_kt_interleaved(0)
            s1_epi_act(0, 0)
            s1_epi_act(1, 0)
            # img0 stage 2: m1 first so its psum frees early for s1(img1)
            s2_m(1, 0)
            s2_taps(1, 0, range(9))
            s2_epi(1, 0)
            s2_m(0, 0)
            s2_taps(0, 0, range(9))
            s2_epi(0, 0)
            # img1 stage 1: m-sequential (m0 chain, epi, m1 chain)
            s1_mms_m_seq(1)
            s1_epi_act(1, 1)
            # img0 stage 3 head starts while img1 epilogues settle
            s3_unit(0, 0)
            s3_unit(1, 0)
            s3_unit(2, 0)
            # img1 stage 2 with img0 stage-3 units interleaved
            s2_m(0, 1)
            s2_taps(0, 1, range(0, 4))
            s3_unit(3, 0)
            s2_taps(0, 1, range(4, 7))
            s3_unit(4, 0)
            s2_taps(0, 1, range(7, 9))
            s2_epi(0, 1)
            s3_unit(5, 0)
            s2_m(1, 1)
            s2_taps(1, 1, range(0, 4))
            s3_unit(6, 0)
            s2_taps(1, 1, range(4, 7))
            s3_unit(7, 0)
            s2_taps(1, 1, range(7, 9))
            s2_epi(1, 1)
            # img1 stage 3
            for m in range(8):
                s3_unit(m, 1)

    nc.compile()
    return nc


def _get_nc():
    if "nc" not in _CACHE:
        _CACHE["nc"] = _build()
    return _CACHE["nc"]


def _pack_inputs(inputs):
    """Host-side: effective weights, bias folds, per-core shards, casts."""
    f32 = np.float32
    f64 = np.float64
    x = np.asarray(inputs["x"])

    def eff(w2, s):
        return (np.asarray(w2, dtype=f32) *
                np.exp2(np.asarray(s).astype(f32))).astype(f32)

    w1e = eff(inputs["w2_1"], inputs["s1"])[:, :, 0, 0]          # [O=256, I=1024]
    w1 = np.ascontiguousarray(
        w1e.T.reshape(8, 128, 2, 128).transpose(1, 0, 2, 3)
        .reshape(128, 16, 128)).astype(F16)
    w2e = eff(inputs["w2_2"], inputs["s2"])                      # [O, I, 3, 3]
    taps = np.stack([w2e[:, :, dy, dx].T
                     for dy in range(3) for dx in range(3)])     # [9, I, O]
    w2 = np.ascontiguousarray(
        taps.reshape(9, 2, 128, 2, 128)
        .transpose(2, 0, 3, 1, 4)
        .reshape(128, 18, 2, 128)).astype(FP8)
    w3e = eff(inputs["w2_3"], inputs["s3"])[:, :, 0, 0]          # [O=1024, I=256]
    w3 = np.ascontiguousarray(
        w3e.T.reshape(2, 128, 8, 128)
        .transpose(1, 2, 0, 3)).astype(FP8)

    scl = np.exp2(f32(-12.0))
    b3p = (np.asarray(inputs["beta3"], dtype=f32) *
           np.exp2(np.asarray(inputs["q3"]).astype(f32)))        # [1024]
    a1f = np.asarray(inputs["alpha1"], dtype=f32) * scl
    b1f = (np.asarray(inputs["beta1"], dtype=f32) *
           np.exp2(np.asarray(inputs["q1"]).astype(f32)))
    # stage-1 bias correction for the beta3 folded into xb
    corr = w1e.astype(f64) @ b3p.astype(f64)                     # [256]
    b1c = (b1f.astype(f64) - a1f.astype(f64) * corr).astype(f32)

    vec = np.zeros((128, 24), dtype=f32)
    for m in range(2):
        sl = slice(m * 128, (m + 1) * 128)
        vec[:, m] = a1f[sl]
        vec[:, 2 + m] = b1c[sl]
        vec[:, 4 + m] = np.asarray(inputs["alpha2"], dtype=f32)[sl] * scl
        vec[:, 6 + m] = (np.asarray(inputs["beta2"], dtype=f32)[sl] *
                         np.exp2(np.asarray(inputs["q2"]).astype(f32)[sl]))
    for m in range(8):
        sl = slice(m * 128, (m + 1) * 128)
        vec[:, 8 + m] = np.asarray(inputs["alpha3"], dtype=f32)[sl] * scl

    for k in range(8):
        vec[:, 16 + k] = b3p[k * 128:(k + 1) * 128]

    xb = x.astype(f32) + b3p[None, :, None, None]                # [16,1024,28,28]
    in_maps = []
    for c in range(N_CORES):
        csl = slice(c * N_PER_CORE, (c + 1) * N_PER_CORE)
        x8c = np.ascontiguousarray(
            x[csl][0].reshape(8, 128, HW1)
            .transpose(1, 0, 2)).astype(np.int8)                 # [128,8,784]
        xbc = np.ascontiguousarray(
            xb[csl][1].reshape(8, 128, HW1)
            .transpose(1, 0, 2)).astype(F16)                     # [128,8,784]
        in_maps.append({"x8": x8c, "xb": xbc, "w1": w1, "w2": w2, "w3": w3,
                        "vec": vec})
    return in_maps


def _assemble(results):
    outs = []
    for c in range(N_CORES):
        o = results[c]["out"]                                    # [128,8,1568] i8
        o = np.maximum(o, 0).astype(np.float32)                  # final relu
        # [128p, 8kt, 2img*784] -> [2, 1024, 28, 28]
        o = o.reshape(128, 8, 2, 28, 28).transpose(2, 1, 0, 3, 4).reshape(
            N_PER_CORE, 1024, 28, 28)
        outs.append(o)
    return np.concatenate(outs, axis=0)


def _run(inputs, trace=False, **kwargs):
    from concourse.bass_utils import run_bass_kernel_spmd
    nc = _get_nc()
    in_maps = _pack_inputs(inputs)
    res = run_bass_kernel_spmd(nc, in_maps, list(range(N_CORES)),
                               trace=trace, **kwargs)
    return _assemble(res.results), res


def kernel(**inputs):
    out, _ = _run(inputs)
    return out
